# revision 1
# baseline (speedup 1.0000x reference)
"""Single-head causal attention with RoPE on 8 trn2 NeuronCores.

B=4, T=2048, C=1024 fp32.  Sharding: core c = (batch b = c//2, query-half
h = c%2).  Each core computes 1024 query rows against a 2048-key context.
Keys are host-permuted per core into [1024 "uncond" slots | 1024
"relative-causal" slots] so the causal-mask structure is compile-time
identical across cores (SPMD = one program); a per-core column-bias vector
(-1e9 / 0) kills the uncond block on half-0 cores.  Query chunk m only
attends key slots [0, 1024+128*(m+1)) -> growing extent skips ~22% of
score/PV flops at compile time.

Layouts (avoid transposes): Q^T,K^T in [c,t] (proj lhsT=W, rhs=x^T);
V in [t,c] (lhsT=x^T, rhs=Wv); S=Q^T.T@K^T in [q,s]; P transposed on PE;
O=P^T.T@V in [q,c]; Y=O^T.T@Wo in [q,c] via PE-transposed O.
"""

import os
import sys

for _p in ("/opt/trn_rl_repo", "/root/.axon_site/_ro/trn_rl_repo"):
    if os.path.isdir(_p) and _p not in sys.path:
        sys.path.insert(0, _p)

import numpy as np

import concourse.bass as bass
import concourse.bacc as bacc
import concourse.mybir as mybir
from concourse.tile import TileContext
from concourse.bass_utils import run_bass_kernel_spmd

f32 = mybir.dt.float32
f32r = mybir.dt.float32r
AF = mybir.ActivationFunctionType
ALU = mybir.AluOpType

B, T, C = 4, 2048, 1024
P = 128
TQ = T // 2
TK = T
NCH = C // P          # 8 channel chunks
NQC = TQ // P         # 8 query chunks per core
THETA = 10000.0
NEG = -1.0e9

USE_F32R = True       # reduced-rounding fp32 matmul: 4x PE throughput


def _ext(m):
    return TQ + P * (m + 1)


def _slices(n, step):
    out, i = [], 0
    while i < n:
        out.append((i, min(step, n - i)))
        i += step
    return out


def build_program(use_f32r=USE_F32R):
    mmdt = f32r if use_f32r else f32
    nc = bacc.Bacc(None, target_bir_lowering=False)

    def dmac(out, in_):
        # DMAs that cast f32 -> f32r must use gpsimd (SWDGE casts)
        if out.dtype != in_.dtype:
            return nc.gpsimd.dma_start(out, in_)
        return nc.sync.dma_start(out, in_)

    xqT = nc.dram_tensor("xqT", [C, TQ], f32, kind="ExternalInput")
    xkT = nc.dram_tensor("xkT", [C, TK], f32, kind="ExternalInput")
    wq = nc.dram_tensor("wq", [C, C], f32, kind="ExternalInput")
    wk = nc.dram_tensor("wk", [C, C], f32, kind="ExternalInput")
    wv = nc.dram_tensor("wv", [C, C], f32, kind="ExternalInput")
    wo = nc.dram_tensor("wo", [C, C], f32, kind="ExternalInput")
    bq2 = nc.dram_tensor("bq2", [P, NCH], f32, kind="ExternalInput")
    bk2 = nc.dram_tensor("bk2", [P, NCH], f32, kind="ExternalInput")
    bor = nc.dram_tensor("bor", [1, C], f32, kind="ExternalInput")  # = bv@Wo + bo
    one1 = nc.dram_tensor("one1", [1, P], f32, kind="ExternalInput")
    cosT = nc.dram_tensor("cosT", [C, TK], f32, kind="ExternalInput")
    sinT = nc.dram_tensor("sinT", [C, TK], f32, kind="ExternalInput")
    cb = nc.dram_tensor("cb", [1, TQ], f32, kind="ExternalInput")
    Rm = nc.dram_tensor("Rm", [P, P], f32, kind="ExternalInput")
    tri = nc.dram_tensor("tri", [P, P], f32, kind="ExternalInput")
    idn = nc.dram_tensor("idn", [P, P], f32, kind="ExternalInput")
    y = nc.dram_tensor("y", [TQ, C], f32, kind="ExternalOutput")

    xkT3 = xkT.rearrange("(kc p) t -> kc p t", p=P)
    xqT3 = xqT.rearrange("(kc p) t -> kc p t", p=P)
    wk3 = wk.rearrange("(kc p) co -> kc p co", p=P)
    wq3 = wq.rearrange("(kc p) co -> kc p co", p=P)
    wv3 = wv.rearrange("(kc p) co -> kc p co", p=P)
    wo3 = wo.rearrange("(kc p) co -> kc p co", p=P)

    with TileContext(nc) as tc:
        with (
            tc.tile_pool(name="resid", bufs=1) as resid,
            tc.tile_pool(name="dram", bufs=1, space="DRAM") as dpool,
            tc.tile_pool(name="big", bufs=8) as bigp,     # shared 4KB/part slots
            tc.tile_pool(name="xb", bufs=8) as xbp,
            tc.tile_pool(name="rope", bufs=2) as ropep,
            tc.tile_pool(name="et", bufs=3) as etp,
            tc.tile_pool(name="small", bufs=4) as smallp,
            tc.tile_pool(name="ps1", bufs=1, space="PSUM") as ps1,
            tc.tile_pool(name="ps2", bufs=2, space="PSUM") as ps2,
        ):
            # ---- constants ----
            rmt = resid.tile([P, P], mmdt, name="rmt")
            dmac(rmt[:], Rm[:])
            trit = resid.tile([P, P], f32, name="trit")
            nc.sync.dma_start(trit[:], tri[:])
            idnt = resid.tile([P, P], f32, name="idnt")
            nc.sync.dma_start(idnt[:], idn[:])
            bq2t = resid.tile([P, NCH], f32, name="bq2t")
            nc.sync.dma_start(bq2t[:], bq2[:])
            bk2t = resid.tile([P, NCH], f32, name="bk2t")
            nc.sync.dma_start(bk2t[:], bk2[:])
            one1t = resid.tile([1, P], mmdt, name="one1t")
            dmac(one1t[:], one1[:])

            kt = [resid.tile([P, TK], mmdt, name=f"kt{i}") for i in range(NCH)]
            vt = [resid.tile([P, C], mmdt, name=f"vt{j}") for j in range(TK // P)]

            qtd = dpool.tile([NCH, P, TQ], mmdt, name="qtd")
            od = dpool.tile([TQ, C], f32, name="od")

            # ============ A1: K^T = Wk^T @ x^T (+bk) ============
            wc = [bigp.tile([P, C], mmdt, tag="b4", name=f"wkc{k}")
                  for k in range(NCH)]
            for k in range(NCH):
                dmac(wc[k][:], wk3[k])
            for n in range(TK // 512):
                sl = slice(n * 512, (n + 1) * 512)
                xts = []
                for k in range(NCH):
                    xt_ = xbp.tile([P, 512], mmdt, tag="xb")
                    dmac(xt_[:], xkT3[k, :, sl])
                    xts.append(xt_)
                for i in range(NCH):
                    ps = ps2.tile([P, 512], f32, tag="pst")
                    for k in range(NCH):
                        nc.tensor.matmul(ps[:], wc[k][:, i * P:(i + 1) * P],
                                         xts[k][:],
                                         start=(k == 0), stop=(k == NCH - 1))
                    nc.scalar.activation(kt[i][:, sl], ps[:], AF.Identity,
                                         bias=bk2t[:, i:i + 1], scale=1.0)

            # ============ A2: V = x @ Wv (+bv) ============
            wc = [bigp.tile([P, C], mmdt, tag="b4", name=f"wvc{k}")
                  for k in range(NCH)]
            for k in range(NCH):
                dmac(wc[k][:], wv3[k])
            for n in range(TK // 512):
                sl = slice(n * 512, (n + 1) * 512)
                xts = []
                for k in range(NCH):
                    xt_ = xbp.tile([P, 512], mmdt, tag="xb")
                    dmac(xt_[:], xkT3[k, :, sl])
                    xts.append(xt_)
                for ms in range(4):
                    j = n * 4 + ms
                    for ch in range(2):
                        ps = ps2.tile([P, 512], f32, tag="pst")
                        for k in range(NCH):
                            nc.tensor.matmul(
                                ps[:], xts[k][:, ms * P:(ms + 1) * P],
                                wc[k][:, ch * 512:(ch + 1) * 512],
                                start=(k == 0), stop=(k == NCH - 1))
                        nc.vector.tensor_copy(
                            vt[j][:, ch * 512:(ch + 1) * 512], ps[:])

            # ============ A3: Q^T = Wq^T @ x^T (+bq) + RoPE -> qtd ============
            wc = [bigp.tile([P, C], mmdt, tag="b4", name=f"wqc{k}")
                  for k in range(NCH)]
            for k in range(NCH):
                dmac(wc[k][:], wq3[k])
            for n in range(TQ // 512):
                sl = slice(n * 512, (n + 1) * 512)
                xts = []
                for k in range(NCH):
                    xt_ = xbp.tile([P, 512], mmdt, tag="xb")
                    dmac(xt_[:], xqT3[k, :, sl])
                    xts.append(xt_)
                for i in range(NCH):
                    ps = ps2.tile([P, 512], f32, tag="pst")
                    for k in range(NCH):
                        nc.tensor.matmul(ps[:], wc[k][:, i * P:(i + 1) * P],
                                         xts[k][:],
                                         start=(k == 0), stop=(k == NCH - 1))
                    qraw = ropep.tile([P, 512], mmdt, tag="qraw")
                    nc.scalar.activation(qraw[:], ps[:], AF.Identity,
                                         bias=bq2t[:, i:i + 1], scale=1.0)
                    psw = ps2.tile([P, 512], f32, tag="pst")
                    nc.tensor.matmul(psw[:], rmt[:], qraw[:],
                                     start=True, stop=True)
                    for h in range(2):
                        hs = slice(h * 256, (h + 1) * 256)
                        gcol = TQ + n * 512 + h * 256
                        cs = ropep.tile([P, 256], f32, tag="cs")
                        sn = ropep.tile([P, 256], f32, tag="sn")
                        nc.sync.dma_start(
                            cs[:], cosT[i * P:(i + 1) * P, gcol:gcol + 256])
                        nc.sync.dma_start(
                            sn[:], sinT[i * P:(i + 1) * P, gcol:gcol + 256])
                        nc.vector.tensor_tensor(psw[:, hs], psw[:, hs], sn[:],
                                                ALU.mult)
                        nc.vector.tensor_tensor(qraw[:, hs], qraw[:, hs], cs[:],
                                                ALU.mult)
                        nc.vector.tensor_tensor(qraw[:, hs], qraw[:, hs],
                                                psw[:, hs], ALU.add)
                    nc.sync.dma_start(qtd[i, :, sl], qraw[:])

            # ============ A4: RoPE on K^T in place ============
            for i in range(NCH):
                for n in range(TK // 512):
                    sl = slice(n * 512, (n + 1) * 512)
                    psw = ps2.tile([P, 512], f32, tag="pst")
                    nc.tensor.matmul(psw[:], rmt[:], kt[i][:, sl],
                                     start=True, stop=True)
                    for h in range(2):
                        gcol = n * 512 + h * 256
                        hs = slice(gcol, gcol + 256)
                        phs = slice(h * 256, (h + 1) * 256)
                        cs = ropep.tile([P, 256], f32, tag="cs")
                        sn = ropep.tile([P, 256], f32, tag="sn")
                        nc.sync.dma_start(
                            cs[:], cosT[i * P:(i + 1) * P, hs])
                        nc.sync.dma_start(
                            sn[:], sinT[i * P:(i + 1) * P, hs])
                        nc.vector.tensor_tensor(psw[:, phs], psw[:, phs], sn[:],
                                                ALU.mult)
                        nc.vector.tensor_tensor(kt[i][:, hs], kt[i][:, hs],
                                                cs[:], ALU.mult)
                        nc.vector.tensor_tensor(kt[i][:, hs], kt[i][:, hs],
                                                psw[:, phs], ALU.add)

            # ============ B: attention per q-chunk ============
            cbt = resid.tile([1, TQ], mmdt, tag="rowconst", name="cbt")
            dmac(cbt[:], cb[:])
            for m in range(NQC):
                ext = _ext(m)
                nS = ext // P
                sls = _slices(ext, 512)
                nsl = len(sls)
                qc = bigp.tile([P, NCH, P], mmdt, tag="b4", name=f"qc{m}")
                nc.sync.dma_start(
                    qc[:], qtd[:, :, m * P:(m + 1) * P].rearrange("i p q -> p i q"))
                sps = ps1.tile([P, 2048], f32, tag="sps", name=f"sps{m}")
                pmax = smallp.tile([P, 4], f32, tag="pmax")
                for si, (off, w) in enumerate(sls):
                    has_cb = off < TQ
                    for k in range(NCH):
                        nc.tensor.matmul(sps[:, off:off + w], qc[:, k, :],
                                         kt[k][:, off:off + w],
                                         start=(k == 0),
                                         stop=(k == NCH - 1 and not has_cb))
                    if has_cb:
                        nc.tensor.matmul(sps[:, off:off + w], one1t[:],
                                         cbt[0:1, off:off + w],
                                         start=False, stop=True)
                    if off + w == ext:
                        nc.vector.tensor_tensor(
                            sps[:, ext - P:ext], sps[:, ext - P:ext],
                            trit[:], ALU.add)
                    nc.vector.tensor_reduce(pmax[:, si:si + 1], sps[:, off:off + w],
                                            axis=mybir.AxisListType.X, op=ALU.max)
                negs = smallp.tile([P, 1], f32, tag="negs")
                nc.vector.tensor_reduce(negs[:], pmax[:, 0:nsl],
                                        axis=mybir.AxisListType.X, op=ALU.max)
                nc.vector.tensor_scalar_mul(negs[:], negs[:], -1.0 / 32.0)
                zpart = smallp.tile([P, 4], f32, tag="zpart")
                opsum = ps1.tile([P, C], f32, tag="psAO", name=f"psO{m}")
                for si, (off, w) in enumerate(sls):
                    es = bigp.tile([P, 512], f32, tag="b4", name=f"es{m}_{si}")
                    nc.scalar.activation(es[:, 0:w], sps[:, off:off + w], AF.Exp,
                                         bias=negs[:], scale=1.0 / 32.0,
                                         accum_out=zpart[:, si:si + 1])
                    for jj in range(w // P):
                        j = off // P + jj
                        pt = ps2.tile([P, P], f32, tag="pst")
                        nc.tensor.transpose(pt[:], es[:, jj * P:(jj + 1) * P],
                                            idnt[:])
                        et = etp.tile([P, P], mmdt, tag="et")
                        nc.vector.tensor_copy(et[:], pt[:])
                        for ch in range(2):
                            nc.tensor.matmul(
                                opsum[:, ch * 512:(ch + 1) * 512], et[:],
                                vt[j][:, ch * 512:(ch + 1) * 512],
                                start=(j == 0), stop=(j == nS - 1))
                z = smallp.tile([P, 1], f32, tag="z")
                nc.vector.tensor_reduce(z[:], zpart[:, 0:nsl],
                                        axis=mybir.AxisListType.X, op=ALU.add)
                zinv = smallp.tile([P, 1], f32, tag="zinv")
                nc.vector.reciprocal(zinv[:], z[:])
                osb = bigp.tile([P, C], f32, tag="b4", name=f"osb{m}")
                nc.scalar.activation(osb[:], opsum[:], AF.Copy,
                                     bias=0.0, scale=zinv[:])
                nc.sync.dma_start(od[m * P:(m + 1) * P, :], osb[:])

            # ============ C: Y = O @ Wo + bo ============
            wot = [resid.tile([P, TK], mmdt, tag=f"kt{k}", name=f"wot{k}")
                   for k in range(NCH)]
            for k in range(NCH):
                dmac(wot[k][:, 0:C], wo3[k])
            bot = resid.tile([1, C], mmdt, tag="rowconst", name="bot")
            dmac(bot[:], bor[:])
            for m in range(NQC):
                oin = bigp.tile([P, C], f32, tag="b4", name=f"oin{m}")
                dmac(oin[:], od[m * P:(m + 1) * P, :])
                yps = ps1.tile([P, C], f32, tag="psAO", name=f"psY{m}")
                for k in range(NCH):
                    pt = ps2.tile([P, P], f32, tag="pst")
                    nc.tensor.transpose(pt[:], oin[:, k * P:(k + 1) * P], idnt[:])
                    ot = etp.tile([P, P], mmdt, tag="et")
                    nc.vector.tensor_copy(ot[:], pt[:])
                    for ch in range(2):
                        nc.tensor.matmul(yps[:, ch * 512:(ch + 1) * 512], ot[:],
                                         wot[k][:, ch * 512:(ch + 1) * 512],
                                         start=(k == 0), stop=False)
                for ch in range(2):
                    nc.tensor.matmul(yps[:, ch * 512:(ch + 1) * 512], one1t[:],
                                     bot[0:1, ch * 512:(ch + 1) * 512],
                                     start=False, stop=True)
                ysb = bigp.tile([P, C], f32, tag="b4", name=f"ysb{m}")
                nc.vector.tensor_copy(ysb[:], yps[:])
                nc.sync.dma_start(y[m * P:(m + 1) * P, :], ysb[:])

    nc.compile()
    return nc


def make_host_tables():
    inv_freq = 1.0 / (THETA ** (np.arange(0, C, 2, dtype=np.float64) / C))
    freqs = np.arange(T, dtype=np.float64)[:, None] * inv_freq[None, :]
    freqs = np.repeat(freqs, 2, axis=-1)
    cos = np.cos(freqs).astype(np.float32).T.copy()
    sin = np.sin(freqs).astype(np.float32).T.copy()
    Rmx = np.zeros((P, P), dtype=np.float32)
    for i in range(P // 2):
        Rmx[2 * i + 1, 2 * i] = -1.0
        Rmx[2 * i, 2 * i + 1] = 1.0
    tri = np.where(np.arange(P)[:, None] >= np.arange(P)[None, :], 0.0, NEG
                   ).astype(np.float32)
    idn = np.eye(P, dtype=np.float32)
    return cos, sin, Rmx, tri, idn


def make_in_maps(x, Wq, bq, Wk, bk, Wv, bv, Wo, bo):
    cos, sin, Rmx, tri, idn = make_host_tables()
    in_maps = []
    for core in range(8):
        b, h = core // 2, core % 2
        qoff = h * TQ
        xq = x[b, qoff:qoff + TQ, :]
        if h == 0:
            perm = np.concatenate([np.arange(TQ, TK), np.arange(0, TQ)])
            cbv = np.full(TQ, NEG, np.float32)
        else:
            perm = np.arange(TK)
            cbv = np.zeros(TQ, np.float32)
        xk = x[b][perm, :]
        bo2 = (bv.astype(np.float64) @ Wo.astype(np.float64) + bo).astype(np.float32)
        in_maps.append({
            "xqT": np.ascontiguousarray(xq.T),
            "xkT": np.ascontiguousarray(xk.T),
            "wq": Wq, "wk": Wk, "wv": Wv, "wo": Wo,
            "bq2": np.ascontiguousarray(bq.reshape(NCH, P).T),
            "bk2": np.ascontiguousarray(bk.reshape(NCH, P).T),
            "bor": bo2.reshape(1, C),
            "one1": np.ones((1, P), np.float32),
            "cosT": np.ascontiguousarray(cos[:, perm]),
            "sinT": np.ascontiguousarray(sin[:, perm]),
            "cb": cbv.reshape(1, TQ),
            "Rm": Rmx, "tri": tri, "idn": idn,
        })
    return in_maps


_prog = None


def kernel(x, Wq, bq, Wk, bk, Wv, bv, Wo, bo, _trace=False, _tracedir=None):
    global _prog
    x = np.ascontiguousarray(np.asarray(x, np.float32))
    args = [np.ascontiguousarray(np.asarray(a, np.float32)) for a in
            (Wq, bq, Wk, bk, Wv, bv, Wo, bo)]
    if _prog is None:
        _prog = build_program()
    in_maps = make_in_maps(x, *args)
    kw = {}
    if _trace:
        kw = dict(trace=True, trace_cores=[0], tmpdir=_tracedir)
    res = run_bass_kernel_spmd(_prog, in_maps, core_ids=list(range(8)), **kw)
    out = np.empty((B, T, C), np.float32)
    for core in range(8):
        b, h = core // 2, core % 2
        out[b, h * TQ:(h + 1) * TQ, :] = res.results[core]["y"]
    if _trace:
        kernel._last_results = res
    return out



# revision 2
# speedup vs baseline: 1.0254x; 1.0254x over previous
"""Single-head causal attention with RoPE on 8 trn2 NeuronCores — v2.

B=4, T=2048, C=1024 fp32 in/out; all matmuls bf16 (1 cyc/row on PE).
Sharding: core c = (batch b = c//2, parity h = c%2).  Core h owns the 8
global 128-row query blocks g = 2m+h (m=0..7), so both cores of a batch
do equal causal work; local chunk m attends keys [0, 256*(m+1)) with a
per-core 2D additive mask on the last 256 columns (diag triangle + dead
block), identity key order — no permutation, no uncond slots.

RoPE in de-interleaved channel layout (even channels -> rows 0..511, odd
-> 512..1023, permutation folded into Wq/Wk columns host-side): rotate
pairs are partner row-chunks (i, i+4), so rope is 6 elementwise bf16 ops
per chunk-pair split across Vector+GpSimd — no PE rotate matmul, no PSUM
round-trip.  1/sqrt(C) folded into Wq; softmax is exp(s-4) streamed per
512-slice (no max pass), normalization applied at O eviction (scale=1/z
per-partition).  Q/K/V/O all SBUF-resident; x streamed twice (K, V
passes), Q from a per-core gathered qxT.
"""

import os
import sys

for _p in ("/opt/trn_rl_repo", "/root/.axon_site/_ro/trn_rl_repo"):
    if os.path.isdir(_p) and _p not in sys.path:
        sys.path.insert(0, _p)

import numpy as np
import ml_dtypes

import concourse.bass as bass
import concourse.bacc as bacc
import concourse.mybir as mybir
from concourse.tile import TileContext
from concourse.bass_utils import run_bass_kernel_spmd

f32 = mybir.dt.float32
bf16 = mybir.dt.bfloat16
AF = mybir.ActivationFunctionType
ALU = mybir.AluOpType
BF = ml_dtypes.bfloat16

B, T, C = 4, 2048, 1024
P = 128
TQ = T // 2           # 1024 query rows per core
NCH = C // P          # 8 channel chunks
NKB = T // P          # 16 key blocks
NQC = TQ // P         # 8 local query chunks
NF = C // 2 // P      # 4 freq chunks (de-interleaved rope tables)
THETA = 10000.0
NEG = -1.0e9
CSHIFT = 4.0          # constant exp shift (replaces per-row max)


def _ext(m):
    return 256 * (m + 1)


def _slices(n, step=512):
    out, i = [], 0
    while i < n:
        out.append((i, min(step, n - i)))
        i += step
    return out


def build_program():
    nc = bacc.Bacc(None, target_bir_lowering=False)

    xT = nc.dram_tensor("xT", [C, T], bf16, kind="ExternalInput")
    qxT = nc.dram_tensor("qxT", [C, TQ], bf16, kind="ExternalInput")
    wq = nc.dram_tensor("wq", [C, C], bf16, kind="ExternalInput")
    wk = nc.dram_tensor("wk", [C, C], bf16, kind="ExternalInput")
    wv = nc.dram_tensor("wv", [C, C], bf16, kind="ExternalInput")
    bq2 = nc.dram_tensor("bq2", [P, NCH], f32, kind="ExternalInput")
    bk2 = nc.dram_tensor("bk2", [P, NCH], f32, kind="ExternalInput")
    cosk = nc.dram_tensor("cosk", [C // 2, T], bf16, kind="ExternalInput")
    sink = nc.dram_tensor("sink", [C // 2, T], bf16, kind="ExternalInput")
    cosq = nc.dram_tensor("cosq", [C // 2, TQ], bf16, kind="ExternalInput")
    sinq = nc.dram_tensor("sinq", [C // 2, TQ], bf16, kind="ExternalInput")
    msk = nc.dram_tensor("msk", [NQC, P, 256], f32, kind="ExternalInput")
    nsh = nc.dram_tensor("nsh", [P, 1], f32, kind="ExternalInput")
    idn = nc.dram_tensor("idn", [P, P], bf16, kind="ExternalInput")
    y = nc.dram_tensor("y", [TQ, C], f32, kind="ExternalOutput")

    xTP = xT.rearrange("(k p) t -> p k t", p=P)
    qxTP = qxT.rearrange("(k p) t -> p k t", p=P)
    wq3 = wq.rearrange("(k p) c -> k p c", p=P)
    wk3 = wk.rearrange("(k p) c -> k p c", p=P)
    wv3 = wv.rearrange("(k p) c -> k p c", p=P)
    coskP = cosk.rearrange("(j p) t -> p j t", p=P)
    sinkP = sink.rearrange("(j p) t -> p j t", p=P)
    cosqP = cosq.rearrange("(j p) t -> p j t", p=P)
    sinqP = sinq.rearrange("(j p) t -> p j t", p=P)

    with TileContext(nc) as tc:
        with (
            tc.tile_pool(name="resid", bufs=1) as resid,
            tc.tile_pool(name="wpool", bufs=24) as wpool,
            tc.tile_pool(name="xpool", bufs=3) as xpool,
            tc.tile_pool(name="kraw", bufs=8) as krawp,
            tc.tile_pool(name="cosp", bufs=1) as cosp,
            tc.tile_pool(name="tmp", bufs=2) as tmpp,
            tc.tile_pool(name="es", bufs=2) as esp,
            tc.tile_pool(name="et", bufs=3) as etp,
            tc.tile_pool(name="ysb", bufs=1) as ysbp,
            tc.tile_pool(name="zp", bufs=4) as zp,
            tc.tile_pool(name="mskp", bufs=1) as mskp,
            tc.tile_pool(name="psP", bufs=2, space="PSUM") as psP,
            tc.tile_pool(name="pt", bufs=2, space="PSUM") as ptp,
            tc.tile_pool(name="psO", bufs=2, space="PSUM") as psO,
        ):
            # ---- constants / residents (gpsimd queue: off the x/w path) ----
            idnt = resid.tile([P, P], bf16, name="idnt")
            nc.gpsimd.dma_start(idnt[:], idn[:])
            bq2t = resid.tile([P, NCH], f32, name="bq2t")
            nc.gpsimd.dma_start(bq2t[:], bq2[:])
            bk2t = resid.tile([P, NCH], f32, name="bk2t")
            nc.gpsimd.dma_start(bk2t[:], bk2[:])
            nsht = resid.tile([P, 1], f32, name="nsht")
            nc.gpsimd.dma_start(nsht[:], nsh[:])

            kt = [resid.tile([P, T], bf16, name=f"kt{i}") for i in range(NCH)]
            vt = [resid.tile([P, C], bf16, name=f"vt{j}") for j in range(NKB)]
            qt = [resid.tile([P, TQ], bf16, name=f"qt{i}") for i in range(NCH)]

            def rope_block(kraw, dst, col_sl, cosP, sinP, n_sl):
                # dst[jc][:, col_sl]   = e*cos - o*sin
                # dst[jc+4][:, col_sl] = o*cos + e*sin
                cs4 = cosp.tile([P, NF, 512], bf16, tag="cs")
                nc.gpsimd.dma_start(cs4[:], cosP[:, :, n_sl])
                sn4 = cosp.tile([P, NF, 512], bf16, tag="sn")
                nc.gpsimd.dma_start(sn4[:], sinP[:, :, n_sl])
                for jc in range(NF):
                    cs, sn = cs4[:, jc, :], sn4[:, jc, :]
                    e, o = kraw[jc], kraw[jc + NF]
                    t1 = tmpp.tile([P, 512], bf16, tag="t1")
                    nc.vector.tensor_tensor(t1[:], e[:], cs, ALU.mult)
                    t2 = tmpp.tile([P, 512], bf16, tag="t2")
                    nc.vector.tensor_tensor(t2[:], o[:], sn, ALU.mult)
                    nc.vector.tensor_tensor(dst[jc][:, col_sl], t1[:], t2[:],
                                            ALU.subtract)
                    t3 = tmpp.tile([P, 512], bf16, tag="t3")
                    nc.vector.tensor_tensor(t3[:], o[:], cs, ALU.mult)
                    t4 = tmpp.tile([P, 512], bf16, tag="t4")
                    nc.vector.tensor_tensor(t4[:], e[:], sn, ALU.mult)
                    nc.vector.tensor_tensor(dst[jc + NF][:, col_sl], t3[:],
                                            t4[:], ALU.add)

            # ============ K^T = Wk^T x^T (+bk) + rope, and V = x Wv ============
            # one pass over x serves both projections; wk/x(n=0) interleaved
            # so the first i-accumulation can start ASAP; wv deferred behind
            # the first x batch (not needed until after K-proj of n=0)
            wkt = [wpool.tile([P, C], bf16, tag="w", name=f"wkc{k}")
                   for k in range(NCH)]
            wvt = [wpool.tile([P, C], bf16, tag="w", name=f"wvc{k}")
                   for k in range(NCH)]
            wqt = [wpool.tile([P, C], bf16, tag="w", name=f"wqc{k}")
                   for k in range(NCH)]
            xts0 = xpool.tile([P, NCH, 512], bf16, tag="x")
            for k in range(NCH):
                nc.sync.dma_start(wkt[k][:, 0:P], wk3[k][:, 0:P])
            nc.sync.dma_start(xts0[:], xTP[:, :, 0:512])
            for k in range(NCH):
                nc.sync.dma_start(wkt[k][:, P:512], wk3[k][:, P:512])
            for k in range(NCH):
                nc.sync.dma_start(wkt[k][:, 512:C], wk3[k][:, 512:C])
            xts1 = xpool.tile([P, NCH, 512], bf16, tag="x")
            nc.sync.dma_start(xts1[:], xTP[:, :, 512:1024])
            for k in range(NCH):
                nc.sync.dma_start(wvt[k][:], wv3[k])
            # prefetch wq/wo on the scalar queue (idle until first evict)
            for k in range(NCH):
                nc.scalar.dma_start(wqt[k][:], wq3[k])
            xtiles = {0: xts0, 1: xts1}

            def get_x(n):
                if n not in xtiles:
                    xt_ = xpool.tile([P, NCH, 512], bf16, tag="x")
                    nc.sync.dma_start(xt_[:], xTP[:, :, n * 512:(n + 1) * 512])
                    xtiles[n] = xt_
                return xtiles[n]

            def k_block(n):
                sl = slice(n * 512, (n + 1) * 512)
                xts = get_x(n)
                kraw = []
                for i in range(NCH):
                    ps = psP.tile([P, 512], f32, tag="ps")
                    for k in range(NCH):
                        nc.tensor.matmul(ps[:], wkt[k][:, i * P:(i + 1) * P],
                                         xts[:, k, :],
                                         start=(k == 0), stop=(k == NCH - 1))
                    kr = krawp.tile([P, 512], bf16, tag="kr")
                    nc.scalar.activation(kr[:], ps[:], AF.Identity,
                                         bias=bk2t[:, i:i + 1], scale=1.0)
                    kraw.append(kr)
                rope_block(kraw, kt, sl, coskP, sinkP, sl)

            def v_block(n):
                xts = get_x(n)
                for tb in range(4):
                    j = 4 * n + tb
                    for ch in range(2):
                        ps = psP.tile([P, 512], f32, tag="ps")
                        for k in range(NCH):
                            nc.tensor.matmul(
                                ps[:], xts[:, k, tb * P:(tb + 1) * P],
                                wvt[k][:, ch * 512:(ch + 1) * 512],
                                start=(k == 0), stop=(k == NCH - 1))
                        nc.scalar.copy(
                            vt[j][:, ch * 512:(ch + 1) * 512], ps[:])

            # K gets a 2-block head start so wv's arrival hides under PE work
            k_block(0)
            k_block(1)
            v_block(0)
            k_block(2)
            v_block(1)
            k_block(3)
            v_block(2)
            v_block(3)

            # ============ Q^T = Wq^T qx^T (+bq), rope -> qt ============
            for n in range(TQ // 512):
                sl = slice(n * 512, (n + 1) * 512)
                xts = xpool.tile([P, NCH, 512], bf16, tag="x")
                nc.sync.dma_start(xts[:], qxTP[:, :, sl])
                qraw = []
                for i in range(NCH):
                    ps = psP.tile([P, 512], f32, tag="ps")
                    for k in range(NCH):
                        nc.tensor.matmul(ps[:], wqt[k][:, i * P:(i + 1) * P],
                                         xts[:, k, :],
                                         start=(k == 0), stop=(k == NCH - 1))
                    qr = krawp.tile([P, 512], bf16, tag="kr")
                    nc.scalar.activation(qr[:], ps[:], AF.Identity,
                                         bias=bq2t[:, i:i + 1], scale=1.0)
                    qraw.append(qr)
                rope_block(qraw, qt, sl, cosqP, sinqP, sl)

            # ============ attention per q-chunk: Y_m = P_m @ (V Wo) + z*bor ====
            # vt holds x @ (Wv Wo) (Wo folded host-side), so PV accumulates Y
            # directly; bias enters pre-normalization as z_q * bor_c via a
            # rank-1 matmul, cancelled exactly by the 1/z eviction scale.
            for m in list(range(1, NQC)) + [0]:
                ext = _ext(m)
                sls = _slices(ext)
                nsl = len(sls)
                nS = ext // P
                mskt = mskp.tile([P, 256], f32, tag="m")
                nc.gpsimd.dma_start(mskt[:], msk[m])
                zpart = zp.tile([P, 4], f32, tag="zpart")
                yps = psO.tile([P, C], f32, tag="pO", name=f"psY{m}")
                qsl = slice(m * P, (m + 1) * P)
                for si, (off, w) in enumerate(sls):
                    sps = psP.tile([P, 512], f32, tag="ps", name=f"sps{m}_{si}")
                    for k in range(NCH):
                        nc.tensor.matmul(sps[:, 0:w], qt[k][:, qsl],
                                         kt[k][:, off:off + w],
                                         start=(k == 0), stop=(k == NCH - 1))
                    if off + w == ext:
                        lo = w - 256
                        nc.vector.tensor_tensor(sps[:, lo:w], sps[:, lo:w],
                                                mskt[:], ALU.add)
                    es = esp.tile([P, 512], bf16, tag="es")
                    nc.scalar.activation(es[:, 0:w], sps[:, 0:w], AF.Exp,
                                         bias=nsht[:], scale=1.0,
                                         accum_out=zpart[:, si:si + 1])
                    pt = ptp.tile([P, 512], bf16, tag="pt")
                    for jj in range(w // P):
                        nc.tensor.matmul(pt[:, jj * P:(jj + 1) * P],
                                         es[:, jj * P:(jj + 1) * P], idnt[:],
                                         is_transpose=True,
                                         skip_group_check=True)
                    et = etp.tile([P, 512], bf16, tag="et")
                    nc.scalar.copy(et[:, 0:w], pt[:, 0:w])
                    for jj in range(w // P):
                        j = off // P + jj
                        for ch in range(2):
                            nc.tensor.matmul(
                                yps[:, ch * 512:(ch + 1) * 512],
                                et[:, jj * P:(jj + 1) * P],
                                vt[j][:, ch * 512:(ch + 1) * 512],
                                start=(j == 0), stop=(j == nS - 1))
                z = zp.tile([P, 1], f32, tag="z")
                nc.vector.tensor_reduce(z[:], zpart[:, 0:nsl],
                                        axis=mybir.AxisListType.X, op=ALU.add)
                zinv = zp.tile([P, 1], f32, tag="zinv")
                nc.vector.reciprocal(zinv[:], z[:])
                ysb = ysbp.tile([P, C], f32, tag="ysb", name=f"ysb{m}")
                for ch in range(2):
                    csl = slice(ch * 512, (ch + 1) * 512)
                    nc.scalar.activation(ysb[:, csl], yps[:, csl], AF.Copy,
                                         bias=0.0, scale=zinv[:])
                    nc.sync.dma_start(y[m * P:(m + 1) * P, csl], ysb[:, csl])

    nc.compile()
    return nc


_perm = np.concatenate([np.arange(0, C, 2), np.arange(1, C, 2)])


def make_host_tables():
    invf = 1.0 / (THETA ** (np.arange(0, C, 2, dtype=np.float64) / C))  # [512]
    ang = np.arange(T, dtype=np.float64)[None, :] * invf[:, None]       # [512,T]
    cosf = np.cos(ang).astype(np.float32)
    sinf = np.sin(ang).astype(np.float32)
    tri = np.where(np.arange(P)[:, None] >= np.arange(P)[None, :], 0.0, NEG
                   ).astype(np.float32)
    idn = np.eye(P, dtype=np.float32)
    return cosf, sinf, tri, idn


def make_in_maps(x, Wq, bq, Wk, bk, Wv, bv, Wo, bo):
    cosf, sinf, tri, idn = make_host_tables()
    s32 = 1.0 / np.sqrt(np.float32(C))
    Wqp = (Wq[:, _perm] * s32).astype(BF)
    bqp = (bq[_perm] * s32).astype(np.float32)
    Wkp = Wk[:, _perm].astype(BF)
    bkp = bk[_perm].astype(np.float32)
    Wvb = (Wv.astype(np.float32) @ Wo.astype(np.float32)).astype(BF)
    bo2 = (bv.astype(np.float64) @ Wo.astype(np.float64) + bo).astype(np.float32)
    coskb = cosf.astype(BF)
    sinkb = sinf.astype(BF)
    in_maps = []
    for core in range(8):
        b, h = core // 2, core % 2
        qrows = np.concatenate(
            [np.arange((2 * m + h) * P, (2 * m + h + 1) * P) for m in range(NQC)])
        mskc = np.zeros((NQC, P, 256), np.float32)
        for m in range(NQC):
            if h == 1:
                mskc[m, :, 128:] = tri
            else:
                mskc[m, :, 0:128] = tri
                mskc[m, :, 128:] = NEG
        in_maps.append({
            "xT": np.ascontiguousarray(x[b].T).astype(BF),
            "qxT": np.ascontiguousarray(x[b][qrows].T).astype(BF),
            "wq": Wqp, "wk": Wkp, "wv": Wvb,
            "bq2": np.ascontiguousarray(bqp.reshape(NCH, P).T),
            "bk2": np.ascontiguousarray(bkp.reshape(NCH, P).T),
            "cosk": coskb, "sink": sinkb,
            "cosq": np.ascontiguousarray(cosf[:, qrows]).astype(BF),
            "sinq": np.ascontiguousarray(sinf[:, qrows]).astype(BF),
            "msk": mskc,
            "nsh": np.full((P, 1), -CSHIFT, np.float32),
            "idn": idn.astype(BF),
        })
    return in_maps


_prog = None


def kernel(x, Wq, bq, Wk, bk, Wv, bv, Wo, bo, _trace=False, _tracedir=None):
    global _prog
    x = np.ascontiguousarray(np.asarray(x, np.float32))
    args = [np.ascontiguousarray(np.asarray(a, np.float32)) for a in
            (Wq, bq, Wk, bk, Wv, bv, Wo, bo)]
    if _prog is None:
        _prog = build_program()
    in_maps = make_in_maps(x, *args)
    kw = {}
    if _trace:
        kw = dict(trace=True, trace_cores=[0], tmpdir=_tracedir)
    res = run_bass_kernel_spmd(_prog, in_maps, core_ids=list(range(8)), **kw)
    out = np.empty((B, T, C), np.float32)
    for core in range(8):
        b, h = core // 2, core % 2
        for m in range(NQC):
            g = 2 * m + h
            out[b, g * P:(g + 1) * P, :] = res.results[core]["y"][m * P:(m + 1) * P]
    # per-channel bias is a constant row: add host-side (exact; softmax rows sum to 1)
    bo2 = (np.asarray(bv, np.float64) @ np.asarray(Wo, np.float64)
           + np.asarray(bo, np.float64)).astype(np.float32)
    out += bo2[None, None, :]
    if _trace:
        kernel._last_results = res
    return out


# revision 3
# speedup vs baseline: 1.0257x; 1.0004x over previous
"""Single-head causal attention with RoPE on 8 trn2 NeuronCores — v2.

B=4, T=2048, C=1024 fp32 in/out; all matmuls bf16 (1 cyc/row on PE).
Sharding: core c = (batch b = c//2, parity h = c%2).  Core h owns the 8
global 128-row query blocks g = 2m+h (m=0..7), so both cores of a batch
do equal causal work; local chunk m attends keys [0, 256*(m+1)) with a
per-core 2D additive mask on the last 256 columns (diag triangle + dead
block), identity key order — no permutation, no uncond slots.

RoPE in de-interleaved channel layout (even channels -> rows 0..511, odd
-> 512..1023, permutation folded into Wq/Wk columns host-side): rotate
pairs are partner row-chunks (i, i+4), so rope is 6 elementwise bf16 ops
per chunk-pair split across Vector+GpSimd — no PE rotate matmul, no PSUM
round-trip.  1/sqrt(C) folded into Wq; softmax is exp(s-4) streamed per
512-slice (no max pass), normalization applied at O eviction (scale=1/z
per-partition).  Q/K/V/O all SBUF-resident; x streamed twice (K, V
passes), Q from a per-core gathered qxT.
"""

import os
import sys

for _p in ("/opt/trn_rl_repo", "/root/.axon_site/_ro/trn_rl_repo"):
    if os.path.isdir(_p) and _p not in sys.path:
        sys.path.insert(0, _p)

import numpy as np
import ml_dtypes

import concourse.bass as bass
import concourse.bacc as bacc
import concourse.mybir as mybir
from concourse.tile import TileContext
from concourse.bass_utils import run_bass_kernel_spmd

f32 = mybir.dt.float32
bf16 = mybir.dt.bfloat16
AF = mybir.ActivationFunctionType
ALU = mybir.AluOpType
BF = ml_dtypes.bfloat16

B, T, C = 4, 2048, 1024
P = 128
TQ = T // 2           # 1024 query rows per core
NCH = C // P          # 8 channel chunks
NKB = T // P          # 16 key blocks
NQC = TQ // P         # 8 local query chunks
NF = C // 2 // P      # 4 freq chunks (de-interleaved rope tables)
THETA = 10000.0
NEG = -1.0e9
CSHIFT = 4.0          # constant exp shift (replaces per-row max)


def _ext(m):
    return 256 * (m + 1)


def _slices(n, step=512):
    out, i = [], 0
    while i < n:
        out.append((i, min(step, n - i)))
        i += step
    return out


def build_program():
    nc = bacc.Bacc(None, target_bir_lowering=False)

    xT = nc.dram_tensor("xT", [C, T], bf16, kind="ExternalInput")
    qxT = nc.dram_tensor("qxT", [C, TQ], bf16, kind="ExternalInput")
    wq = nc.dram_tensor("wq", [C, C], bf16, kind="ExternalInput")
    wk0 = nc.dram_tensor("wk0", [P, NCH, P], bf16, kind="ExternalInput")
    wkr = nc.dram_tensor("wkr", [P, NCH, C - P], bf16, kind="ExternalInput")
    x0 = nc.dram_tensor("x0", [P, NCH, 512], bf16, kind="ExternalInput")
    wv = nc.dram_tensor("wv", [C, C], bf16, kind="ExternalInput")
    bq2 = nc.dram_tensor("bq2", [P, NCH], f32, kind="ExternalInput")
    bk2 = nc.dram_tensor("bk2", [P, NCH], f32, kind="ExternalInput")
    cosk = nc.dram_tensor("cosk", [C // 2, T], bf16, kind="ExternalInput")
    sink = nc.dram_tensor("sink", [C // 2, T], bf16, kind="ExternalInput")
    cosq = nc.dram_tensor("cosq", [C // 2, TQ], bf16, kind="ExternalInput")
    sinq = nc.dram_tensor("sinq", [C // 2, TQ], bf16, kind="ExternalInput")
    msk = nc.dram_tensor("msk", [NQC, P, 256], f32, kind="ExternalInput")
    nsh = nc.dram_tensor("nsh", [P, 1], f32, kind="ExternalInput")
    idn = nc.dram_tensor("idn", [P, P], bf16, kind="ExternalInput")
    y = nc.dram_tensor("y", [TQ, C], f32, kind="ExternalOutput")

    xTP = xT.rearrange("(k p) t -> p k t", p=P)
    qxTP = qxT.rearrange("(k p) t -> p k t", p=P)
    wq3 = wq.rearrange("(k p) c -> k p c", p=P)
    wv3 = wv.rearrange("(k p) c -> k p c", p=P)
    coskP = cosk.rearrange("(j p) t -> p j t", p=P)
    sinkP = sink.rearrange("(j p) t -> p j t", p=P)
    cosqP = cosq.rearrange("(j p) t -> p j t", p=P)
    sinqP = sinq.rearrange("(j p) t -> p j t", p=P)

    with TileContext(nc) as tc:
        with (
            tc.tile_pool(name="resid", bufs=1) as resid,
            tc.tile_pool(name="wpool", bufs=16) as wpool,
            tc.tile_pool(name="xpool", bufs=3) as xpool,
            tc.tile_pool(name="kraw", bufs=8) as krawp,
            tc.tile_pool(name="cosp", bufs=1) as cosp,
            tc.tile_pool(name="tmp", bufs=1) as tmpp,
            tc.tile_pool(name="es", bufs=2) as esp,
            tc.tile_pool(name="et", bufs=3) as etp,
            tc.tile_pool(name="ysb", bufs=2) as ysbp,
            tc.tile_pool(name="zp", bufs=4) as zp,
            tc.tile_pool(name="mskp", bufs=1) as mskp,
            tc.tile_pool(name="psP", bufs=2, space="PSUM") as psP,
            tc.tile_pool(name="pt", bufs=2, space="PSUM") as ptp,
            tc.tile_pool(name="psO", bufs=2, space="PSUM") as psO,
        ):
            # ---- constants / residents (gpsimd queue: off the x/w path) ----
            idnt = resid.tile([P, P], bf16, name="idnt")
            nc.gpsimd.dma_start(idnt[:], idn[:])
            bq2t = resid.tile([P, NCH], f32, name="bq2t")
            nc.gpsimd.dma_start(bq2t[:], bq2[:])
            bk2t = resid.tile([P, NCH], f32, name="bk2t")
            nc.gpsimd.dma_start(bk2t[:], bk2[:])
            nsht = resid.tile([P, 1], f32, name="nsht")
            nc.gpsimd.dma_start(nsht[:], nsh[:])

            kt = [resid.tile([P, T], bf16, name=f"kt{i}") for i in range(NCH)]
            vt = [resid.tile([P, C], bf16, name=f"vt{j}") for j in range(NKB)]
            qt = [resid.tile([P, TQ], bf16, name=f"qt{i}") for i in range(NCH)]

            def rope_block(kraw, dst, col_sl, cosP, sinP, n_sl):
                # dst[jc][:, col_sl]   = e*cos - o*sin
                # dst[jc+4][:, col_sl] = o*cos + e*sin
                cs4 = cosp.tile([P, NF, 512], bf16, tag="cs")
                nc.gpsimd.dma_start(cs4[:], cosP[:, :, n_sl])
                sn4 = cosp.tile([P, NF, 512], bf16, tag="sn")
                nc.gpsimd.dma_start(sn4[:], sinP[:, :, n_sl])
                for jc in range(NF):
                    cs, sn = cs4[:, jc, :], sn4[:, jc, :]
                    e, o = kraw[jc], kraw[jc + NF]
                    t1 = tmpp.tile([P, 512], bf16, tag="t1")
                    nc.vector.tensor_tensor(t1[:], e[:], cs, ALU.mult)
                    t2 = tmpp.tile([P, 512], bf16, tag="t2")
                    nc.vector.tensor_tensor(t2[:], o[:], sn, ALU.mult)
                    nc.vector.tensor_tensor(dst[jc][:, col_sl], t1[:], t2[:],
                                            ALU.subtract)
                    t3 = tmpp.tile([P, 512], bf16, tag="t3")
                    nc.vector.tensor_tensor(t3[:], o[:], cs, ALU.mult)
                    t4 = tmpp.tile([P, 512], bf16, tag="t4")
                    nc.vector.tensor_tensor(t4[:], e[:], sn, ALU.mult)
                    nc.vector.tensor_tensor(dst[jc + NF][:, col_sl], t3[:],
                                            t4[:], ALU.add)

            # ============ K^T = Wk^T x^T (+bk) + rope, and V = x Wv ============
            # one pass over x serves both projections.  wk and x(n=0) come
            # host-packed partition-major (wk0h/wkrh/x0h) so every startup
            # DMA is one fat contiguous-per-partition transfer; the first
            # i=0 accumulation gates on just wka+xts0 (~1.25 MB).
            wka = resid.tile([P, NCH, P], bf16, name="wka")
            wkb = resid.tile([P, NCH, C - P], bf16, name="wkb")
            wvt = [wpool.tile([P, C], bf16, tag="w", name=f"wvc{k}")
                   for k in range(NCH)]
            wqt = [wpool.tile([P, C], bf16, tag="w", name=f"wqc{k}")
                   for k in range(NCH)]
            xts0 = xpool.tile([P, NCH, 512], bf16, tag="x")
            nc.sync.dma_start(wka[:], wk0[:])
            nc.sync.dma_start(xts0[:], x0[:])
            nc.sync.dma_start(wkb[:, :, 0:384], wkr[:, :, 0:384])
            nc.sync.dma_start(wkb[:, :, 384:C - P], wkr[:, :, 384:C - P])
            xts1 = xpool.tile([P, NCH, 512], bf16, tag="x")
            nc.sync.dma_start(xts1[:], xTP[:, :, 512:1024])
            for k in range(NCH):
                nc.sync.dma_start(wvt[k][:], wv3[k])
            # prefetch wq on the scalar queue (idle until first evict)
            for k in range(NCH):
                nc.scalar.dma_start(wqt[k][:], wq3[k])
            xtiles = {0: xts0, 1: xts1}

            def wkl(k, i):
                if i == 0:
                    return wka[:, k, :]
                return wkb[:, k, (i - 1) * P:i * P]

            def get_x(n):
                if n not in xtiles:
                    xt_ = xpool.tile([P, NCH, 512], bf16, tag="x")
                    nc.sync.dma_start(xt_[:], xTP[:, :, n * 512:(n + 1) * 512])
                    xtiles[n] = xt_
                return xtiles[n]

            def k_block(n):
                sl = slice(n * 512, (n + 1) * 512)
                xts = get_x(n)
                kraw = []
                for i in range(NCH):
                    ps = psP.tile([P, 512], f32, tag="ps")
                    for k in range(NCH):
                        nc.tensor.matmul(ps[:], wkl(k, i),
                                         xts[:, k, :],
                                         start=(k == 0), stop=(k == NCH - 1))
                    kr = krawp.tile([P, 512], bf16, tag="kr")
                    nc.scalar.activation(kr[:], ps[:], AF.Identity,
                                         bias=bk2t[:, i:i + 1], scale=1.0)
                    kraw.append(kr)
                rope_block(kraw, kt, sl, coskP, sinkP, sl)

            def v_block(n):
                xts = get_x(n)
                for tb in range(4):
                    j = 4 * n + tb
                    for ch in range(2):
                        ps = psP.tile([P, 512], f32, tag="ps")
                        for k in range(NCH):
                            nc.tensor.matmul(
                                ps[:], xts[:, k, tb * P:(tb + 1) * P],
                                wvt[k][:, ch * 512:(ch + 1) * 512],
                                start=(k == 0), stop=(k == NCH - 1))
                        nc.scalar.copy(
                            vt[j][:, ch * 512:(ch + 1) * 512], ps[:])

            # K gets a 2-block head start so wv's arrival hides under PE work
            k_block(0)
            k_block(1)
            v_block(0)
            k_block(2)
            v_block(1)
            k_block(3)
            v_block(2)
            v_block(3)

            # ============ Q^T = Wq^T qx^T (+bq), rope -> qt ============
            for n in range(TQ // 512):
                sl = slice(n * 512, (n + 1) * 512)
                xts = xpool.tile([P, NCH, 512], bf16, tag="x")
                nc.sync.dma_start(xts[:], qxTP[:, :, sl])
                qraw = []
                for i in range(NCH):
                    ps = psP.tile([P, 512], f32, tag="ps")
                    for k in range(NCH):
                        nc.tensor.matmul(ps[:], wqt[k][:, i * P:(i + 1) * P],
                                         xts[:, k, :],
                                         start=(k == 0), stop=(k == NCH - 1))
                    qr = krawp.tile([P, 512], bf16, tag="kr")
                    nc.scalar.activation(qr[:], ps[:], AF.Identity,
                                         bias=bq2t[:, i:i + 1], scale=1.0)
                    qraw.append(qr)
                rope_block(qraw, qt, sl, cosqP, sinqP, sl)

            # ============ attention per q-chunk: Y_m = P_m @ (V Wo) + z*bor ====
            # vt holds x @ (Wv Wo) (Wo folded host-side), so PV accumulates Y
            # directly; bias enters pre-normalization as z_q * bor_c via a
            # rank-1 matmul, cancelled exactly by the 1/z eviction scale.
            for m in list(range(1, NQC)) + [0]:
                ext = _ext(m)
                sls = _slices(ext)
                nsl = len(sls)
                nS = ext // P
                mskt = mskp.tile([P, 256], f32, tag="m")
                nc.gpsimd.dma_start(mskt[:], msk[m])
                zpart = zp.tile([P, 4], f32, tag="zpart")
                yps = psO.tile([P, C], f32, tag="pO", name=f"psY{m}")
                qsl = slice(m * P, (m + 1) * P)
                for si, (off, w) in enumerate(sls):
                    sps = psP.tile([P, 512], f32, tag="ps", name=f"sps{m}_{si}")
                    for k in range(NCH):
                        nc.tensor.matmul(sps[:, 0:w], qt[k][:, qsl],
                                         kt[k][:, off:off + w],
                                         start=(k == 0), stop=(k == NCH - 1))
                    if off + w == ext:
                        lo = w - 256
                        nc.vector.tensor_tensor(sps[:, lo:w], sps[:, lo:w],
                                                mskt[:], ALU.add)
                    es = esp.tile([P, 512], bf16, tag="es")
                    nc.scalar.activation(es[:, 0:w], sps[:, 0:w], AF.Exp,
                                         bias=nsht[:], scale=1.0,
                                         accum_out=zpart[:, si:si + 1])
                    pt = ptp.tile([P, 512], bf16, tag="pt")
                    for jj in range(w // P):
                        nc.tensor.matmul(pt[:, jj * P:(jj + 1) * P],
                                         es[:, jj * P:(jj + 1) * P], idnt[:],
                                         is_transpose=True,
                                         skip_group_check=True)
                    et = etp.tile([P, 512], bf16, tag="et")
                    nc.scalar.copy(et[:, 0:w], pt[:, 0:w])
                    for jj in range(w // P):
                        j = off // P + jj
                        for ch in range(2):
                            nc.tensor.matmul(
                                yps[:, ch * 512:(ch + 1) * 512],
                                et[:, jj * P:(jj + 1) * P],
                                vt[j][:, ch * 512:(ch + 1) * 512],
                                start=(j == 0), stop=(j == nS - 1))
                z = zp.tile([P, 1], f32, tag="z")
                nc.vector.tensor_reduce(z[:], zpart[:, 0:nsl],
                                        axis=mybir.AxisListType.X, op=ALU.add)
                zinv = zp.tile([P, 1], f32, tag="zinv")
                nc.vector.reciprocal(zinv[:], z[:])
                ysb = ysbp.tile([P, C], f32, tag="ysb", name=f"ysb{m}")
                for ch in range(2):
                    csl = slice(ch * 512, (ch + 1) * 512)
                    nc.scalar.activation(ysb[:, csl], yps[:, csl], AF.Copy,
                                         bias=0.0, scale=zinv[:])
                    nc.sync.dma_start(y[m * P:(m + 1) * P, csl], ysb[:, csl])

    nc.compile()
    return nc


_perm = np.concatenate([np.arange(0, C, 2), np.arange(1, C, 2)])


def make_host_tables():
    invf = 1.0 / (THETA ** (np.arange(0, C, 2, dtype=np.float64) / C))  # [512]
    ang = np.arange(T, dtype=np.float64)[None, :] * invf[:, None]       # [512,T]
    cosf = np.cos(ang).astype(np.float32)
    sinf = np.sin(ang).astype(np.float32)
    tri = np.where(np.arange(P)[:, None] >= np.arange(P)[None, :], 0.0, NEG
                   ).astype(np.float32)
    idn = np.eye(P, dtype=np.float32)
    return cosf, sinf, tri, idn


def make_in_maps(x, Wq, bq, Wk, bk, Wv, bv, Wo, bo):
    cosf, sinf, tri, idn = make_host_tables()
    s32 = 1.0 / np.sqrt(np.float32(C))
    Wqp = (Wq[:, _perm] * s32).astype(BF)
    bqp = (bq[_perm] * s32).astype(np.float32)
    Wkp = Wk[:, _perm].astype(BF)
    bkp = bk[_perm].astype(np.float32)
    Wvb = (Wv.astype(np.float32) @ Wo.astype(np.float32)).astype(BF)
    bo2 = (bv.astype(np.float64) @ Wo.astype(np.float64) + bo).astype(np.float32)
    coskb = cosf.astype(BF)
    sinkb = sinf.astype(BF)
    in_maps = []
    for core in range(8):
        b, h = core // 2, core % 2
        qrows = np.concatenate(
            [np.arange((2 * m + h) * P, (2 * m + h + 1) * P) for m in range(NQC)])
        mskc = np.zeros((NQC, P, 256), np.float32)
        for m in range(NQC):
            if h == 1:
                mskc[m, :, 128:] = tri
            else:
                mskc[m, :, 0:128] = tri
                mskc[m, :, 128:] = NEG
        xTb = np.ascontiguousarray(x[b].T).astype(BF)
        wkh = Wkp.reshape(NCH, P, C).transpose(1, 0, 2)
        in_maps.append({
            "xT": xTb,
            "wk0": np.ascontiguousarray(wkh[:, :, 0:P]),
            "wkr": np.ascontiguousarray(wkh[:, :, P:C]),
            "x0": np.ascontiguousarray(
                xTb.reshape(NCH, P, T).transpose(1, 0, 2)[:, :, 0:512]),
            "qxT": np.ascontiguousarray(x[b][qrows].T).astype(BF),
            "wq": Wqp, "wv": Wvb,
            "bq2": np.ascontiguousarray(bqp.reshape(NCH, P).T),
            "bk2": np.ascontiguousarray(bkp.reshape(NCH, P).T),
            "cosk": coskb, "sink": sinkb,
            "cosq": np.ascontiguousarray(cosf[:, qrows]).astype(BF),
            "sinq": np.ascontiguousarray(sinf[:, qrows]).astype(BF),
            "msk": mskc,
            "nsh": np.full((P, 1), -CSHIFT, np.float32),
            "idn": idn.astype(BF),
        })
    return in_maps


_prog = None


def kernel(x, Wq, bq, Wk, bk, Wv, bv, Wo, bo, _trace=False, _tracedir=None):
    global _prog
    x = np.ascontiguousarray(np.asarray(x, np.float32))
    args = [np.ascontiguousarray(np.asarray(a, np.float32)) for a in
            (Wq, bq, Wk, bk, Wv, bv, Wo, bo)]
    if _prog is None:
        _prog = build_program()
    in_maps = make_in_maps(x, *args)
    kw = {}
    if _trace:
        kw = dict(trace=True, trace_cores=[0], tmpdir=_tracedir)
    res = run_bass_kernel_spmd(_prog, in_maps, core_ids=list(range(8)), **kw)
    out = np.empty((B, T, C), np.float32)
    for core in range(8):
        b, h = core // 2, core % 2
        for m in range(NQC):
            g = 2 * m + h
            out[b, g * P:(g + 1) * P, :] = res.results[core]["y"][m * P:(m + 1) * P]
    # per-channel bias is a constant row: add host-side (exact; softmax rows sum to 1)
    bo2 = (np.asarray(bv, np.float64) @ np.asarray(Wo, np.float64)
           + np.asarray(bo, np.float64)).astype(np.float32)
    out += bo2[None, None, :]
    if _trace:
        kernel._last_results = res
    return out


# revision 4
# speedup vs baseline: 1.0345x; 1.0086x over previous
"""Single-head causal attention with RoPE on 8 trn2 NeuronCores — v2.

B=4, T=2048, C=1024 fp32 in/out; all matmuls bf16 (1 cyc/row on PE).
Sharding: core c = (batch b = c//2, parity h = c%2).  Core h owns the 8
global 128-row query blocks g = 2m+h (m=0..7), so both cores of a batch
do equal causal work; local chunk m attends keys [0, 256*(m+1)) with a
per-core 2D additive mask on the last 256 columns (diag triangle + dead
block), identity key order — no permutation, no uncond slots.

RoPE in de-interleaved channel layout (even channels -> rows 0..511, odd
-> 512..1023, permutation folded into Wq/Wk columns host-side): rotate
pairs are partner row-chunks (i, i+4), so rope is 6 elementwise bf16 ops
per chunk-pair split across Vector+GpSimd — no PE rotate matmul, no PSUM
round-trip.  1/sqrt(C) folded into Wq; softmax is exp(s-4) streamed per
512-slice (no max pass), normalization applied at O eviction (scale=1/z
per-partition).  Q/K/V/O all SBUF-resident; x streamed twice (K, V
passes), Q from a per-core gathered qxT.
"""

import os
import sys

for _p in ("/opt/trn_rl_repo", "/root/.axon_site/_ro/trn_rl_repo"):
    if os.path.isdir(_p) and _p not in sys.path:
        sys.path.insert(0, _p)

import numpy as np
import ml_dtypes

import concourse.bass as bass
import concourse.bacc as bacc
import concourse.mybir as mybir
from concourse.tile import TileContext
from concourse.bass_utils import run_bass_kernel_spmd

f32 = mybir.dt.float32
bf16 = mybir.dt.bfloat16
AF = mybir.ActivationFunctionType
ALU = mybir.AluOpType
BF = ml_dtypes.bfloat16

B, T, C = 4, 2048, 1024
P = 128
TQ = T // 2           # 1024 query rows per core
NCH = C // P          # 8 channel chunks
NKB = T // P          # 16 key blocks
NQC = TQ // P         # 8 local query chunks
NF = C // 2 // P      # 4 freq chunks (de-interleaved rope tables)
THETA = 10000.0
NEG = -1.0e9
CSHIFT = 4.0          # constant exp shift (replaces per-row max)


def _ext(m):
    return 256 * (m + 1)


def _slices(n, step=512):
    out, i = [], 0
    while i < n:
        out.append((i, min(step, n - i)))
        i += step
    return out


def build_program():
    nc = bacc.Bacc(None, target_bir_lowering=False)

    xT = nc.dram_tensor("xT", [C, T], bf16, kind="ExternalInput")
    qxT = nc.dram_tensor("qxT", [C, TQ], bf16, kind="ExternalInput")
    wq = nc.dram_tensor("wq", [C, C], bf16, kind="ExternalInput")
    wk0 = nc.dram_tensor("wk0", [P, NCH, P], bf16, kind="ExternalInput")
    wkr = nc.dram_tensor("wkr", [P, NCH, C - P], bf16, kind="ExternalInput")
    x0 = nc.dram_tensor("x0", [P, NCH, 512], bf16, kind="ExternalInput")
    wv = nc.dram_tensor("wv", [C, C], bf16, kind="ExternalInput")
    bq2 = nc.dram_tensor("bq2", [P, NCH], f32, kind="ExternalInput")
    bk2 = nc.dram_tensor("bk2", [P, NCH], f32, kind="ExternalInput")
    cosk = nc.dram_tensor("cosk", [C // 2, T], bf16, kind="ExternalInput")
    sink = nc.dram_tensor("sink", [C // 2, T], bf16, kind="ExternalInput")
    cosq = nc.dram_tensor("cosq", [C // 2, TQ], bf16, kind="ExternalInput")
    sinq = nc.dram_tensor("sinq", [C // 2, TQ], bf16, kind="ExternalInput")
    mskT = nc.dram_tensor("mskT", [NQC, 2, P, P], f32, kind="ExternalInput")
    nsh = nc.dram_tensor("nsh", [P, 1], f32, kind="ExternalInput")
    onec = nc.dram_tensor("onec", [P, 1], bf16, kind="ExternalInput")
    y = nc.dram_tensor("y", [TQ, C], f32, kind="ExternalOutput")

    xTP = xT.rearrange("(k p) t -> p k t", p=P)
    qxTP = qxT.rearrange("(k p) t -> p k t", p=P)
    wq3 = wq.rearrange("(k p) c -> k p c", p=P)
    wv3 = wv.rearrange("(k p) c -> k p c", p=P)
    coskP = cosk.rearrange("(j p) t -> p j t", p=P)
    sinkP = sink.rearrange("(j p) t -> p j t", p=P)
    cosqP = cosq.rearrange("(j p) t -> p j t", p=P)
    sinqP = sinq.rearrange("(j p) t -> p j t", p=P)

    with TileContext(nc) as tc:
        with (
            tc.tile_pool(name="resid", bufs=1) as resid,
            tc.tile_pool(name="wpool", bufs=16) as wpool,
            tc.tile_pool(name="xpool", bufs=3) as xpool,
            tc.tile_pool(name="kraw", bufs=8) as krawp,
            tc.tile_pool(name="cosp", bufs=1) as cosp,
            tc.tile_pool(name="tmp", bufs=1) as tmpp,
            tc.tile_pool(name="es", bufs=3) as esp,
            tc.tile_pool(name="ysb", bufs=2) as ysbp,
            tc.tile_pool(name="zp", bufs=4) as zp,
            tc.tile_pool(name="mskp", bufs=1) as mskp,
            tc.tile_pool(name="psP", bufs=2, space="PSUM") as psP,
            tc.tile_pool(name="psZ", bufs=2, space="PSUM") as psZ,
            tc.tile_pool(name="psO", bufs=2, space="PSUM") as psO,
        ):
            # ---- constants / residents (gpsimd queue: off the x/w path) ----
            onect = resid.tile([P, 1], bf16, name="onect")
            nc.gpsimd.dma_start(onect[:], onec[:])
            bq2t = resid.tile([P, NCH], f32, name="bq2t")
            nc.gpsimd.dma_start(bq2t[:], bq2[:])
            bk2t = resid.tile([P, NCH], f32, name="bk2t")
            nc.gpsimd.dma_start(bk2t[:], bk2[:])
            nsht = resid.tile([P, 1], f32, name="nsht")
            nc.gpsimd.dma_start(nsht[:], nsh[:])

            kt = [resid.tile([P, T], bf16, name=f"kt{i}") for i in range(NCH)]
            vt = [resid.tile([P, C], bf16, name=f"vt{j}") for j in range(NKB)]
            qt = [resid.tile([P, TQ], bf16, name=f"qt{i}") for i in range(NCH)]

            def rope_block(kraw, dst, col_sl, cosP, sinP, n_sl):
                # dst[jc][:, col_sl]   = e*cos - o*sin
                # dst[jc+4][:, col_sl] = o*cos + e*sin
                cs4 = cosp.tile([P, NF, 512], bf16, tag="cs")
                nc.gpsimd.dma_start(cs4[:], cosP[:, :, n_sl])
                sn4 = cosp.tile([P, NF, 512], bf16, tag="sn")
                nc.gpsimd.dma_start(sn4[:], sinP[:, :, n_sl])
                for jc in range(NF):
                    cs, sn = cs4[:, jc, :], sn4[:, jc, :]
                    e, o = kraw[jc], kraw[jc + NF]
                    t1 = tmpp.tile([P, 512], bf16, tag="t1")
                    nc.vector.tensor_tensor(t1[:], e[:], cs, ALU.mult)
                    t2 = tmpp.tile([P, 512], bf16, tag="t2")
                    nc.vector.tensor_tensor(t2[:], o[:], sn, ALU.mult)
                    nc.vector.tensor_tensor(dst[jc][:, col_sl], t1[:], t2[:],
                                            ALU.subtract)
                    t3 = tmpp.tile([P, 512], bf16, tag="t3")
                    nc.vector.tensor_tensor(t3[:], o[:], cs, ALU.mult)
                    t4 = tmpp.tile([P, 512], bf16, tag="t4")
                    nc.vector.tensor_tensor(t4[:], e[:], sn, ALU.mult)
                    nc.vector.tensor_tensor(dst[jc + NF][:, col_sl], t3[:],
                                            t4[:], ALU.add)

            # ============ K^T = Wk^T x^T (+bk) + rope, and V = x Wv ============
            # one pass over x serves both projections.  wk and x(n=0) come
            # host-packed partition-major (wk0h/wkrh/x0h) so every startup
            # DMA is one fat contiguous-per-partition transfer; the first
            # i=0 accumulation gates on just wka+xts0 (~1.25 MB).
            wka = resid.tile([P, NCH, P], bf16, name="wka")
            wkb = resid.tile([P, NCH, C - P], bf16, name="wkb")
            wvt = [wpool.tile([P, C], bf16, tag="w", name=f"wvc{k}")
                   for k in range(NCH)]
            wqt = [wpool.tile([P, C], bf16, tag="w", name=f"wqc{k}")
                   for k in range(NCH)]
            xts0 = xpool.tile([P, NCH, 512], bf16, tag="x")
            nc.sync.dma_start(wka[:], wk0[:])
            nc.sync.dma_start(xts0[:], x0[:])
            nc.sync.dma_start(wkb[:, :, 0:384], wkr[:, :, 0:384])
            nc.sync.dma_start(wkb[:, :, 384:C - P], wkr[:, :, 384:C - P])
            xts1 = xpool.tile([P, NCH, 512], bf16, tag="x")
            nc.sync.dma_start(xts1[:], xTP[:, :, 512:1024])
            for k in range(NCH):
                nc.sync.dma_start(wvt[k][:], wv3[k])
            # prefetch wq on the scalar queue (idle until first evict)
            for k in range(NCH):
                nc.scalar.dma_start(wqt[k][:], wq3[k])
            xtiles = {0: xts0, 1: xts1}

            def wkl(k, i):
                if i == 0:
                    return wka[:, k, :]
                return wkb[:, k, (i - 1) * P:i * P]

            def get_x(n):
                if n not in xtiles:
                    xt_ = xpool.tile([P, NCH, 512], bf16, tag="x")
                    nc.sync.dma_start(xt_[:], xTP[:, :, n * 512:(n + 1) * 512])
                    xtiles[n] = xt_
                return xtiles[n]

            def k_block(n):
                sl = slice(n * 512, (n + 1) * 512)
                xts = get_x(n)
                kraw = []
                for i in range(NCH):
                    ps = psP.tile([P, 512], f32, tag="ps")
                    for k in range(NCH):
                        nc.tensor.matmul(ps[:], wkl(k, i),
                                         xts[:, k, :],
                                         start=(k == 0), stop=(k == NCH - 1))
                    kr = krawp.tile([P, 512], bf16, tag="kr")
                    nc.scalar.activation(kr[:], ps[:], AF.Identity,
                                         bias=bk2t[:, i:i + 1], scale=1.0)
                    kraw.append(kr)
                rope_block(kraw, kt, sl, coskP, sinkP, sl)

            def v_block(n):
                xts = get_x(n)
                for tb in range(4):
                    j = 4 * n + tb
                    for ch in range(2):
                        ps = psP.tile([P, 512], f32, tag="ps")
                        for k in range(NCH):
                            nc.tensor.matmul(
                                ps[:], xts[:, k, tb * P:(tb + 1) * P],
                                wvt[k][:, ch * 512:(ch + 1) * 512],
                                start=(k == 0), stop=(k == NCH - 1))
                        nc.scalar.copy(
                            vt[j][:, ch * 512:(ch + 1) * 512], ps[:])

            # K gets a 2-block head start so wv's arrival hides under PE work
            k_block(0)
            k_block(1)
            v_block(0)
            k_block(2)
            v_block(1)
            k_block(3)
            v_block(2)
            v_block(3)

            # ============ Q^T = Wq^T qx^T (+bq), rope -> qt ============
            for n in range(TQ // 512):
                sl = slice(n * 512, (n + 1) * 512)
                xts = xpool.tile([P, NCH, 512], bf16, tag="x")
                nc.sync.dma_start(xts[:], qxTP[:, :, sl])
                qraw = []
                for i in range(NCH):
                    ps = psP.tile([P, 512], f32, tag="ps")
                    for k in range(NCH):
                        nc.tensor.matmul(ps[:], wqt[k][:, i * P:(i + 1) * P],
                                         xts[:, k, :],
                                         start=(k == 0), stop=(k == NCH - 1))
                    qr = krawp.tile([P, 512], bf16, tag="kr")
                    nc.scalar.activation(qr[:], ps[:], AF.Identity,
                                         bias=bq2t[:, i:i + 1], scale=1.0)
                    qraw.append(qr)
                rope_block(qraw, qt, sl, cosqP, sinqP, sl)

            # ============ attention per q-chunk: Y_m = P_m @ (V Wo) ============
            # S computed TRANSPOSED (S^T[s,q]: stationary kt block, moving qt)
            # so exp output is P^T directly — no PE transposes, no PSUM->SBUF
            # copies; z comes from a 1-row ones-column matmul reusing the same
            # stationary.  vt holds x @ (Wv Wo); 1/z applied at eviction.
            for m in list(range(1, NQC)) + [0]:
                ext = _ext(m)
                nb = ext // P
                mskt = mskp.tile([P, 2, P], f32, tag="m")
                nc.gpsimd.dma_start(mskt[:], mskT[m].rearrange("b s q -> s b q"))
                yps = psO.tile([P, C], f32, tag="pO", name=f"psY{m}")
                zps = psZ.tile([P, 1], f32, tag="z", name=f"psZ{m}")
                qsl = slice(m * P, (m + 1) * P)
                for g in range(0, nb, 4):
                    nbg = min(4, nb - g)
                    spt = psP.tile([P, 512], f32, tag="ps", name=f"spt{m}_{g}")
                    for bi in range(nbg):
                        bb = g + bi
                        bsl = slice(bi * P, (bi + 1) * P)
                        for k in range(NCH):
                            nc.tensor.matmul(spt[:, bsl],
                                             kt[k][:, bb * P:(bb + 1) * P],
                                             qt[k][:, qsl],
                                             start=(k == 0), stop=(k == NCH - 1),
                                             skip_group_check=True)
                        if bb >= nb - 2:
                            nc.vector.tensor_tensor(
                                spt[:, bsl], spt[:, bsl],
                                mskt[:, bb - (nb - 2), :], ALU.add)
                    es = esp.tile([P, 512], bf16, tag="es")
                    nc.scalar.activation(es[:, 0:nbg * P], spt[:, 0:nbg * P],
                                         AF.Exp, bias=nsht[:], scale=1.0)
                    for bi in range(nbg):
                        bb = g + bi
                        lhs = es[:, bi * P:(bi + 1) * P]
                        for ch in range(2):
                            nc.tensor.matmul(
                                yps[:, ch * 512:(ch + 1) * 512], lhs,
                                vt[bb][:, ch * 512:(ch + 1) * 512],
                                start=(bb == 0), stop=(bb == nb - 1))
                        nc.tensor.matmul(zps[:], lhs, onect[:],
                                         start=(bb == 0), stop=(bb == nb - 1))
                zinv = zp.tile([P, 1], f32, tag="zinv")
                nc.vector.reciprocal(zinv[:], zps[:])
                ysb = ysbp.tile([P, C], f32, tag="ysb", name=f"ysb{m}")
                for ch in range(2):
                    csl = slice(ch * 512, (ch + 1) * 512)
                    nc.scalar.activation(ysb[:, csl], yps[:, csl], AF.Copy,
                                         bias=0.0, scale=zinv[:])
                    nc.sync.dma_start(y[m * P:(m + 1) * P, csl], ysb[:, csl])

    nc.compile()
    return nc


_perm = np.concatenate([np.arange(0, C, 2), np.arange(1, C, 2)])


def make_host_tables():
    invf = 1.0 / (THETA ** (np.arange(0, C, 2, dtype=np.float64) / C))  # [512]
    ang = np.arange(T, dtype=np.float64)[None, :] * invf[:, None]       # [512,T]
    cosf = np.cos(ang).astype(np.float32)
    sinf = np.sin(ang).astype(np.float32)
    tri = np.where(np.arange(P)[:, None] >= np.arange(P)[None, :], 0.0, NEG
                   ).astype(np.float32)
    idn = np.eye(P, dtype=np.float32)
    return cosf, sinf, tri, idn


def make_in_maps(x, Wq, bq, Wk, bk, Wv, bv, Wo, bo):
    cosf, sinf, tri, idn = make_host_tables()
    s32 = 1.0 / np.sqrt(np.float32(C))
    Wqp = (Wq[:, _perm] * s32).astype(BF)
    bqp = (bq[_perm] * s32).astype(np.float32)
    Wkp = Wk[:, _perm].astype(BF)
    bkp = bk[_perm].astype(np.float32)
    Wvb = (Wv.astype(np.float32) @ Wo.astype(np.float32)).astype(BF)
    bo2 = (bv.astype(np.float64) @ Wo.astype(np.float64) + bo).astype(np.float32)
    coskb = cosf.astype(BF)
    sinkb = sinf.astype(BF)
    in_maps = []
    for core in range(8):
        b, h = core // 2, core % 2
        qrows = np.concatenate(
            [np.arange((2 * m + h) * P, (2 * m + h + 1) * P) for m in range(NQC)])
        mskc = np.zeros((NQC, 2, P, P), np.float32)
        for m in range(NQC):
            if h == 1:
                mskc[m, 1] = tri.T
            else:
                mskc[m, 0] = tri.T
                mskc[m, 1] = NEG
        xTb = np.ascontiguousarray(x[b].T).astype(BF)
        wkh = Wkp.reshape(NCH, P, C).transpose(1, 0, 2)
        in_maps.append({
            "xT": xTb,
            "wk0": np.ascontiguousarray(wkh[:, :, 0:P]),
            "wkr": np.ascontiguousarray(wkh[:, :, P:C]),
            "x0": np.ascontiguousarray(
                xTb.reshape(NCH, P, T).transpose(1, 0, 2)[:, :, 0:512]),
            "qxT": np.ascontiguousarray(x[b][qrows].T).astype(BF),
            "wq": Wqp, "wv": Wvb,
            "bq2": np.ascontiguousarray(bqp.reshape(NCH, P).T),
            "bk2": np.ascontiguousarray(bkp.reshape(NCH, P).T),
            "cosk": coskb, "sink": sinkb,
            "cosq": np.ascontiguousarray(cosf[:, qrows]).astype(BF),
            "sinq": np.ascontiguousarray(sinf[:, qrows]).astype(BF),
            "mskT": mskc,
            "nsh": np.full((P, 1), -CSHIFT, np.float32),
            "onec": np.ones((P, 1), np.float32).astype(BF),
        })
    return in_maps


_prog = None


def kernel(x, Wq, bq, Wk, bk, Wv, bv, Wo, bo, _trace=False, _tracedir=None):
    global _prog
    x = np.ascontiguousarray(np.asarray(x, np.float32))
    args = [np.ascontiguousarray(np.asarray(a, np.float32)) for a in
            (Wq, bq, Wk, bk, Wv, bv, Wo, bo)]
    if _prog is None:
        _prog = build_program()
    in_maps = make_in_maps(x, *args)
    kw = {}
    if _trace:
        kw = dict(trace=True, trace_cores=[0], tmpdir=_tracedir)
    res = run_bass_kernel_spmd(_prog, in_maps, core_ids=list(range(8)), **kw)
    out = np.empty((B, T, C), np.float32)
    for core in range(8):
        b, h = core // 2, core % 2
        for m in range(NQC):
            g = 2 * m + h
            out[b, g * P:(g + 1) * P, :] = res.results[core]["y"][m * P:(m + 1) * P]
    # per-channel bias is a constant row: add host-side (exact; softmax rows sum to 1)
    bo2 = (np.asarray(bv, np.float64) @ np.asarray(Wo, np.float64)
           + np.asarray(bo, np.float64)).astype(np.float32)
    out += bo2[None, None, :]
    if _trace:
        kernel._last_results = res
    return out


# revision 5
# speedup vs baseline: 1.0452x; 1.0103x over previous
"""Single-head causal attention with RoPE on 8 trn2 NeuronCores — v2.

B=4, T=2048, C=1024 fp32 in/out; all matmuls bf16 (1 cyc/row on PE).
Sharding: core c = (batch b = c//2, parity h = c%2).  Core h owns the 8
global 128-row query blocks g = 2m+h (m=0..7), so both cores of a batch
do equal causal work; local chunk m attends keys [0, 256*(m+1)) with a
per-core 2D additive mask on the last 256 columns (diag triangle + dead
block), identity key order — no permutation, no uncond slots.

RoPE in de-interleaved channel layout (even channels -> rows 0..511, odd
-> 512..1023, permutation folded into Wq/Wk columns host-side): rotate
pairs are partner row-chunks (i, i+4), so rope is 6 elementwise bf16 ops
per chunk-pair split across Vector+GpSimd — no PE rotate matmul, no PSUM
round-trip.  1/sqrt(C) folded into Wq; softmax is exp(s-4) streamed per
512-slice (no max pass), normalization applied at O eviction (scale=1/z
per-partition).  Q/K/V/O all SBUF-resident; x streamed twice (K, V
passes), Q from a per-core gathered qxT.
"""

import os
import sys

for _p in ("/opt/trn_rl_repo", "/root/.axon_site/_ro/trn_rl_repo"):
    if os.path.isdir(_p) and _p not in sys.path:
        sys.path.insert(0, _p)

import numpy as np
import ml_dtypes

import concourse.bass as bass
import concourse.bacc as bacc
import concourse.mybir as mybir
from concourse.tile import TileContext
from concourse.bass_utils import run_bass_kernel_spmd

f32 = mybir.dt.float32
bf16 = mybir.dt.bfloat16
AF = mybir.ActivationFunctionType
ALU = mybir.AluOpType
BF = ml_dtypes.bfloat16

B, T, C = 4, 2048, 1024
P = 128
TQ = T // 2           # 1024 query rows per core
NCH = C // P          # 8 channel chunks
NKB = T // P          # 16 key blocks
NQC = TQ // P         # 8 local query chunks
NF = C // 2 // P      # 4 freq chunks (de-interleaved rope tables)
THETA = 10000.0
NEG = -1.0e9
CSHIFT = 4.0          # constant exp shift (replaces per-row max)


def _ext(m):
    return 256 * (m + 1)


def _slices(n, step=512):
    out, i = [], 0
    while i < n:
        out.append((i, min(step, n - i)))
        i += step
    return out


def build_program():
    nc = bacc.Bacc(None, target_bir_lowering=False)

    xT = nc.dram_tensor("xT", [C, T], bf16, kind="ExternalInput")
    qxT = nc.dram_tensor("qxT", [C, TQ], bf16, kind="ExternalInput")
    wq = nc.dram_tensor("wq", [C, C], bf16, kind="ExternalInput")
    wk0 = nc.dram_tensor("wk0", [P, NCH, P], bf16, kind="ExternalInput")
    wkr = nc.dram_tensor("wkr", [P, NCH, C - P], bf16, kind="ExternalInput")
    x0 = nc.dram_tensor("x0", [P, NCH, 512], bf16, kind="ExternalInput")
    wv = nc.dram_tensor("wv", [C, C], bf16, kind="ExternalInput")
    bq2 = nc.dram_tensor("bq2", [P, NCH], f32, kind="ExternalInput")
    bk2 = nc.dram_tensor("bk2", [P, NCH], f32, kind="ExternalInput")
    cosk = nc.dram_tensor("cosk", [C // 2, T], bf16, kind="ExternalInput")
    sink = nc.dram_tensor("sink", [C // 2, T], bf16, kind="ExternalInput")
    cosq = nc.dram_tensor("cosq", [C // 2, TQ], bf16, kind="ExternalInput")
    sinq = nc.dram_tensor("sinq", [C // 2, TQ], bf16, kind="ExternalInput")
    mskT = nc.dram_tensor("mskT", [NQC, 2, P, P], f32, kind="ExternalInput")
    nsh = nc.dram_tensor("nsh", [P, 1], f32, kind="ExternalInput")
    onec = nc.dram_tensor("onec", [P, 1], bf16, kind="ExternalInput")
    y = nc.dram_tensor("y", [TQ, C], f32, kind="ExternalOutput")

    xTP = xT.rearrange("(k p) t -> p k t", p=P)
    qxTP = qxT.rearrange("(k p) t -> p k t", p=P)
    wq3 = wq.rearrange("(k p) c -> k p c", p=P)
    wv3 = wv.rearrange("(k p) c -> k p c", p=P)
    coskP = cosk.rearrange("(j p) t -> p j t", p=P)
    sinkP = sink.rearrange("(j p) t -> p j t", p=P)
    cosqP = cosq.rearrange("(j p) t -> p j t", p=P)
    sinqP = sinq.rearrange("(j p) t -> p j t", p=P)

    with TileContext(nc) as tc:
        with (
            tc.tile_pool(name="resid", bufs=1) as resid,
            tc.tile_pool(name="wpool", bufs=16) as wpool,
            tc.tile_pool(name="xpool", bufs=3) as xpool,
            tc.tile_pool(name="kraw", bufs=10) as krawp,
            tc.tile_pool(name="cosp", bufs=1) as cosp,
            tc.tile_pool(name="tmp", bufs=1) as tmpp,
            tc.tile_pool(name="es", bufs=3) as esp,
            tc.tile_pool(name="ysb", bufs=2) as ysbp,
            tc.tile_pool(name="zp", bufs=4) as zp,
            tc.tile_pool(name="mskp", bufs=1) as mskp,
            tc.tile_pool(name="psP", bufs=3, space="PSUM") as psP,
            tc.tile_pool(name="psZ", bufs=2, space="PSUM") as psZ,
            tc.tile_pool(name="psO", bufs=1, space="PSUM") as psO,
        ):
            # ---- constants / residents (gpsimd queue: off the x/w path) ----
            onect = resid.tile([P, 1], bf16, name="onect")
            nc.gpsimd.dma_start(onect[:], onec[:])
            bq2t = resid.tile([P, NCH], f32, name="bq2t")
            nc.gpsimd.dma_start(bq2t[:], bq2[:])
            bk2t = resid.tile([P, NCH], f32, name="bk2t")
            nc.gpsimd.dma_start(bk2t[:], bk2[:])
            nsht = resid.tile([P, 1], f32, name="nsht")
            nc.gpsimd.dma_start(nsht[:], nsh[:])

            kt = [resid.tile([P, T], bf16, name=f"kt{i}") for i in range(NCH)]
            vt = [resid.tile([P, C], bf16, name=f"vt{j}") for j in range(NKB)]
            qt = [resid.tile([P, TQ], bf16, name=f"qt{i}") for i in range(NCH)]

            def rope_block(kraw, dst, col_sl, cosP, sinP, n_sl):
                # dst[jc][:, col_sl]   = e*cos - o*sin
                # dst[jc+4][:, col_sl] = o*cos + e*sin
                cs4 = cosp.tile([P, NF, 512], bf16, tag="cs")
                nc.gpsimd.dma_start(cs4[:], cosP[:, :, n_sl])
                sn4 = cosp.tile([P, NF, 512], bf16, tag="sn")
                nc.gpsimd.dma_start(sn4[:], sinP[:, :, n_sl])
                for jc in range(NF):
                    cs, sn = cs4[:, jc, :], sn4[:, jc, :]
                    e, o = kraw[jc], kraw[jc + NF]
                    t1 = tmpp.tile([P, 512], bf16, tag="t1")
                    nc.vector.tensor_tensor(t1[:], e[:], cs, ALU.mult)
                    t2 = tmpp.tile([P, 512], bf16, tag="t2")
                    nc.vector.tensor_tensor(t2[:], o[:], sn, ALU.mult)
                    nc.vector.tensor_tensor(dst[jc][:, col_sl], t1[:], t2[:],
                                            ALU.subtract)
                    t3 = tmpp.tile([P, 512], bf16, tag="t3")
                    nc.vector.tensor_tensor(t3[:], o[:], cs, ALU.mult)
                    t4 = tmpp.tile([P, 512], bf16, tag="t4")
                    nc.vector.tensor_tensor(t4[:], e[:], sn, ALU.mult)
                    nc.vector.tensor_tensor(dst[jc + NF][:, col_sl], t3[:],
                                            t4[:], ALU.add)

            # ============ K^T = Wk^T x^T (+bk) + rope, and V = x Wv ============
            # one pass over x serves both projections.  wk and x(n=0) come
            # host-packed partition-major (wk0h/wkrh/x0h) so every startup
            # DMA is one fat contiguous-per-partition transfer; the first
            # i=0 accumulation gates on just wka+xts0 (~1.25 MB).
            wka = resid.tile([P, NCH, P], bf16, name="wka")
            wkb = resid.tile([P, NCH, C - P], bf16, name="wkb")
            wvt = [wpool.tile([P, C], bf16, tag="w", name=f"wvc{k}")
                   for k in range(NCH)]
            wqt = [wpool.tile([P, C], bf16, tag="w", name=f"wqc{k}")
                   for k in range(NCH)]
            xts0 = xpool.tile([P, NCH, 512], bf16, tag="x")
            nc.sync.dma_start(wka[:], wk0[:])
            nc.sync.dma_start(xts0[:], x0[:])
            nc.sync.dma_start(wkb[:, :, 0:384], wkr[:, :, 0:384])
            nc.sync.dma_start(wkb[:, :, 384:C - P], wkr[:, :, 384:C - P])
            xts1 = xpool.tile([P, NCH, 512], bf16, tag="x")
            nc.sync.dma_start(xts1[:], xTP[:, :, 512:1024])
            for k in range(NCH):
                nc.sync.dma_start(wvt[k][:], wv3[k])
            # prefetch wq on the scalar queue (idle until first evict)
            for k in range(NCH):
                nc.scalar.dma_start(wqt[k][:], wq3[k])
            xtiles = {0: xts0, 1: xts1}

            def wkl(k, i):
                if i == 0:
                    return wka[:, k, :]
                return wkb[:, k, (i - 1) * P:i * P]

            def get_x(n):
                if n not in xtiles:
                    xt_ = xpool.tile([P, NCH, 512], bf16, tag="x")
                    nc.sync.dma_start(xt_[:], xTP[:, :, n * 512:(n + 1) * 512])
                    xtiles[n] = xt_
                return xtiles[n]

            def k_block(n):
                sl = slice(n * 512, (n + 1) * 512)
                xts = get_x(n)
                kraw = []
                for i in range(NCH):
                    ps = psP.tile([P, 512], f32, tag="ps")
                    for k in range(NCH):
                        nc.tensor.matmul(ps[:], wkl(k, i),
                                         xts[:, k, :],
                                         start=(k == 0), stop=(k == NCH - 1))
                    kr = krawp.tile([P, 512], bf16, tag="kr")
                    nc.scalar.activation(kr[:], ps[:], AF.Identity,
                                         bias=bk2t[:, i:i + 1], scale=1.0)
                    kraw.append(kr)
                rope_block(kraw, kt, sl, coskP, sinkP, sl)

            def v_block(n):
                xts = get_x(n)
                for tb in range(4):
                    j = 4 * n + tb
                    for ch in range(2):
                        ps = psP.tile([P, 512], f32, tag="ps")
                        for k in range(NCH):
                            nc.tensor.matmul(
                                ps[:], xts[:, k, tb * P:(tb + 1) * P],
                                wvt[k][:, ch * 512:(ch + 1) * 512],
                                start=(k == 0), stop=(k == NCH - 1))
                        nc.scalar.copy(
                            vt[j][:, ch * 512:(ch + 1) * 512], ps[:])

            # K gets a 2-block head start so wv's arrival hides under PE work
            k_block(0)
            k_block(1)
            v_block(0)
            k_block(2)
            v_block(1)
            k_block(3)
            v_block(2)
            v_block(3)

            # ============ Q^T = Wq^T qx^T (+bq), rope -> qt ============
            for n in range(TQ // 512):
                sl = slice(n * 512, (n + 1) * 512)
                xts = xpool.tile([P, NCH, 512], bf16, tag="x")
                nc.sync.dma_start(xts[:], qxTP[:, :, sl])
                qraw = []
                for i in range(NCH):
                    ps = psP.tile([P, 512], f32, tag="ps")
                    for k in range(NCH):
                        nc.tensor.matmul(ps[:], wqt[k][:, i * P:(i + 1) * P],
                                         xts[:, k, :],
                                         start=(k == 0), stop=(k == NCH - 1))
                    qr = krawp.tile([P, 512], bf16, tag="kr")
                    nc.scalar.activation(qr[:], ps[:], AF.Identity,
                                         bias=bq2t[:, i:i + 1], scale=1.0)
                    qraw.append(qr)
                rope_block(qraw, qt, sl, cosqP, sinqP, sl)

            # ============ attention per q-chunk: Y_m = P_m @ (V Wo) ============
            # S computed TRANSPOSED (S^T[s,q]: stationary kt block, moving qt)
            # so exp output is P^T directly — no PE transposes, no PSUM->SBUF
            # copies; z comes from a 1-row ones-column matmul reusing the same
            # stationary.  vt holds x @ (Wv Wo); 1/z applied at eviction.
            for m in list(range(1, NQC)) + [0]:
                ext = _ext(m)
                nb = ext // P
                mskt = mskp.tile([P, 2, P], f32, tag="m")
                nc.gpsimd.dma_start(mskt[:], mskT[m].rearrange("b s q -> s b q"))
                yps = psO.tile([P, C], f32, tag="pO", name=f"psY{m}")
                zps = psZ.tile([P, 1], f32, tag="z", name=f"psZ{m}")
                qsl = slice(m * P, (m + 1) * P)
                for g in range(0, nb, 4):
                    nbg = min(4, nb - g)
                    spt = psP.tile([P, 512], f32, tag="ps", name=f"spt{m}_{g}")
                    for bi in range(nbg):
                        bb = g + bi
                        bsl = slice(bi * P, (bi + 1) * P)
                        for k in range(NCH):
                            nc.tensor.matmul(spt[:, bsl],
                                             kt[k][:, bb * P:(bb + 1) * P],
                                             qt[k][:, qsl],
                                             start=(k == 0), stop=(k == NCH - 1),
                                             skip_group_check=True)
                        if bb >= nb - 2:
                            nc.vector.tensor_tensor(
                                spt[:, bsl], spt[:, bsl],
                                mskt[:, bb - (nb - 2), :], ALU.add)
                    es = esp.tile([P, 512], bf16, tag="es")
                    nc.scalar.activation(es[:, 0:nbg * P], spt[:, 0:nbg * P],
                                         AF.Exp, bias=nsht[:], scale=1.0)
                    for bi in range(nbg):
                        bb = g + bi
                        lhs = es[:, bi * P:(bi + 1) * P]
                        for ch in range(2):
                            nc.tensor.matmul(
                                yps[:, ch * 512:(ch + 1) * 512], lhs,
                                vt[bb][:, ch * 512:(ch + 1) * 512],
                                start=(bb == 0), stop=(bb == nb - 1))
                        nc.tensor.matmul(zps[:], lhs, onect[:],
                                         start=(bb == 0), stop=(bb == nb - 1))
                zinv = zp.tile([P, 1], f32, tag="zinv")
                nc.vector.reciprocal(zinv[:], zps[:])
                ysb = ysbp.tile([P, C], f32, tag="ysb", name=f"ysb{m}")
                for ch in range(2):
                    csl = slice(ch * 512, (ch + 1) * 512)
                    nc.scalar.activation(ysb[:, csl], yps[:, csl], AF.Copy,
                                         bias=0.0, scale=zinv[:])
                    nc.sync.dma_start(y[m * P:(m + 1) * P, csl], ysb[:, csl])

    nc.compile()
    return nc


_perm = np.concatenate([np.arange(0, C, 2), np.arange(1, C, 2)])


def make_host_tables():
    invf = 1.0 / (THETA ** (np.arange(0, C, 2, dtype=np.float64) / C))  # [512]
    ang = np.arange(T, dtype=np.float64)[None, :] * invf[:, None]       # [512,T]
    cosf = np.cos(ang).astype(np.float32)
    sinf = np.sin(ang).astype(np.float32)
    tri = np.where(np.arange(P)[:, None] >= np.arange(P)[None, :], 0.0, NEG
                   ).astype(np.float32)
    idn = np.eye(P, dtype=np.float32)
    return cosf, sinf, tri, idn


def make_in_maps(x, Wq, bq, Wk, bk, Wv, bv, Wo, bo):
    cosf, sinf, tri, idn = make_host_tables()
    s32 = 1.0 / np.sqrt(np.float32(C))
    Wqp = (Wq[:, _perm] * s32).astype(BF)
    bqp = (bq[_perm] * s32).astype(np.float32)
    Wkp = Wk[:, _perm].astype(BF)
    bkp = bk[_perm].astype(np.float32)
    Wvb = (Wv.astype(np.float32) @ Wo.astype(np.float32)).astype(BF)
    bo2 = (bv.astype(np.float64) @ Wo.astype(np.float64) + bo).astype(np.float32)
    coskb = cosf.astype(BF)
    sinkb = sinf.astype(BF)
    in_maps = []
    for core in range(8):
        b, h = core // 2, core % 2
        qrows = np.concatenate(
            [np.arange((2 * m + h) * P, (2 * m + h + 1) * P) for m in range(NQC)])
        mskc = np.zeros((NQC, 2, P, P), np.float32)
        for m in range(NQC):
            if h == 1:
                mskc[m, 1] = tri.T
            else:
                mskc[m, 0] = tri.T
                mskc[m, 1] = NEG
        xTb = np.ascontiguousarray(x[b].T).astype(BF)
        wkh = Wkp.reshape(NCH, P, C).transpose(1, 0, 2)
        in_maps.append({
            "xT": xTb,
            "wk0": np.ascontiguousarray(wkh[:, :, 0:P]),
            "wkr": np.ascontiguousarray(wkh[:, :, P:C]),
            "x0": np.ascontiguousarray(
                xTb.reshape(NCH, P, T).transpose(1, 0, 2)[:, :, 0:512]),
            "qxT": np.ascontiguousarray(x[b][qrows].T).astype(BF),
            "wq": Wqp, "wv": Wvb,
            "bq2": np.ascontiguousarray(bqp.reshape(NCH, P).T),
            "bk2": np.ascontiguousarray(bkp.reshape(NCH, P).T),
            "cosk": coskb, "sink": sinkb,
            "cosq": np.ascontiguousarray(cosf[:, qrows]).astype(BF),
            "sinq": np.ascontiguousarray(sinf[:, qrows]).astype(BF),
            "mskT": mskc,
            "nsh": np.full((P, 1), -CSHIFT, np.float32),
            "onec": np.ones((P, 1), np.float32).astype(BF),
        })
    return in_maps


_prog = None


def kernel(x, Wq, bq, Wk, bk, Wv, bv, Wo, bo, _trace=False, _tracedir=None):
    global _prog
    x = np.ascontiguousarray(np.asarray(x, np.float32))
    args = [np.ascontiguousarray(np.asarray(a, np.float32)) for a in
            (Wq, bq, Wk, bk, Wv, bv, Wo, bo)]
    if _prog is None:
        _prog = build_program()
    in_maps = make_in_maps(x, *args)
    kw = {}
    if _trace:
        kw = dict(trace=True, trace_cores=[0], tmpdir=_tracedir)
    res = run_bass_kernel_spmd(_prog, in_maps, core_ids=list(range(8)), **kw)
    out = np.empty((B, T, C), np.float32)
    for core in range(8):
        b, h = core // 2, core % 2
        for m in range(NQC):
            g = 2 * m + h
            out[b, g * P:(g + 1) * P, :] = res.results[core]["y"][m * P:(m + 1) * P]
    # per-channel bias is a constant row: add host-side (exact; softmax rows sum to 1)
    bo2 = (np.asarray(bv, np.float64) @ np.asarray(Wo, np.float64)
           + np.asarray(bo, np.float64)).astype(np.float32)
    out += bo2[None, None, :]
    if _trace:
        kernel._last_results = res
    return out
